# revision 9
# baseline (speedup 1.0000x reference)
"""Trainium2 Bass kernel for Bengio03HighwayBiLm.

Model: L=2 layers x 2 directions of [width-4 conv over sequence (H=512 -> 512)
+ ReLU + 2 highway sublayers (512 -> 1024 split into nonlin/gate)].

Sharding: data-parallel over batch across 8 cores (4 batches/core), weights
replicated. On device everything runs in channels-on-partitions layout
([ch, pos]); the host pre-transposes the input / weights and post-transposes
the output, so the device kernel needs no transposes at all. The conv is 4
accumulated matmuls over a column-shifted padded activation window.

Precision: layer 0 runs in float16 (full PE rate). Layer 1 runs its matmuls
in float8e4 with perf_mode=DoubleRow (2 fp8 weights per PE cell -> 2x rate,
K=256 per instruction). fp8 operands carry power-of-2 scales (weights 2^9 /
2^8, activations 2^3) undone for free via the scalar-activation `scale`
operand; PSUM accumulates fp32. End-to-end rel-rms error ~1.2e-2 (layer-0
fp8 would blow the 2e-2 budget; layer-1-only keeps 40% margin).
ScalarE applies bias+ReLU / bias+Sigmoid straight out of PSUM; VectorE does
the 3-op highway combine x' = r + g*(x - r) plus the fp8 re-quant copies.
"""

import sys

for _p in ("/opt/trn_rl_repo", "/root/.axon_site/_ro/trn_rl_repo"):
    if _p not in sys.path:
        sys.path.append(_p)

from contextlib import ExitStack

import numpy as np
import ml_dtypes

import concourse.bass as bass
import concourse.tile as tile
from concourse import bacc, bass_utils, mybir

F32 = mybir.dt.float32
F16 = mybir.dt.float16
F8 = mybir.dt.float8e4
AF = mybir.ActivationFunctionType
DR = mybir.MatmulPerfMode.DoubleRow
E4NP = ml_dtypes.float8_e4m3

B, S, H = 32, 512, 512
L, NHW, WIDTH = 2, 2, 3
NCORES = 8
BL = B // NCORES          # batches per core
SP = S + 2 * WIDTH        # padded sequence length (fp16 layer-0 input)
SP8 = 528                 # fp8 padded length (DoubleRow needs stride % 16 == 0)
HC = H // 128             # channel chunks

# fp8 operand scales (powers of 2; undone in activation `scale`)
XS = 2.0 ** 3             # activations
CWS = 2.0 ** 9            # layer-1 conv weights
HWS = 2.0 ** 8            # layer-1 highway weights

_CACHE = {}


def _build():
    if "nc" in _CACHE:
        return _CACHE["nc"]

    nc = bacc.Bacc("TRN2", target_bir_lowering=False, debug=False,
                   num_devices=NCORES)

    x_t = nc.dram_tensor("x_t", [BL, H, SP], F16, kind="ExternalInput").ap()
    convw = nc.dram_tensor("convw", [2, 4, 128, HC, 512], F16,
                           kind="ExternalInput").ap()          # layer 0
    hww = nc.dram_tensor("hww", [2, NHW, 8, 128, HC, 128], F16,
                         kind="ExternalInput").ap()            # layer 0
    convw8 = nc.dram_tensor("convw8", [2, 4, 2, 128, 2, 512], F8,
                            kind="ExternalInput").ap()         # layer 1
    hww8 = nc.dram_tensor("hww8", [2, NHW, 8, 128, 2, 2, 128], F8,
                          kind="ExternalInput").ap()           # layer 1
    convb = nc.dram_tensor("convb", [L, 2, 128, 4], F32,
                           kind="ExternalInput").ap()
    hwb = nc.dram_tensor("hwb", [L, 2, 128, NHW * 8], F32,
                         kind="ExternalInput").ap()
    padt8 = nc.dram_tensor("padt8", [2, 2, 128, 2, 3], F8,
                           kind="ExternalInput").ap()          # layer-1 pads
    out_t = nc.dram_tensor("out_t", [L, 2, BL, H, S], F16,
                           kind="ExternalOutput").ap()

    with tile.TileContext(nc) as tc, ExitStack() as ctx:
        sb = ctx.enter_context(tc.tile_pool(name="sb", bufs=2))
        ps = ctx.enter_context(tc.tile_pool(name="ps", bufs=8, space="PSUM"))
        dr = ctx.enter_context(tc.tile_pool(name="dr", bufs=1, space="DRAM"))

        # fp8 inter-layer activations, pre-paired for DoubleRow:
        # [b, pair, ki, ko, pos], channel = 128*(2*pair+ko)+ki, value = XS*x
        xmid = []
        for d in range(2):
            m = dr.tile([BL, 2, 128, 2, SP8], F8, name=f"xmid{d}",
                        tag=f"xmid{d}")
            xmid.append(m)

        # ---------------- layer 0 (fp16) ----------------
        for d in range(2):
            def load_xin(b):
                tiles = []
                for hc in range(HC):
                    t = sb.tile([128, SP], F16, name=f"xin_{d}{b}{hc}",
                                tag="xin", bufs=14)
                    nc.sync.dma_start(t[:], x_t[b, hc * 128:(hc + 1) * 128, :])
                    tiles.append(t)
                return tiles

            # interleave first-batch inputs with tap-0 weights so the
            # first matmul's deps land earliest in the DMA queues
            xin0 = []
            wc = [[None] * HC for _ in range(4)]
            for hc in range(HC):
                t = sb.tile([128, SP], F16, name=f"xin_{d}0{hc}",
                            tag="xin", bufs=14)
                nc.sync.dma_start(t[:], x_t[0, hc * 128:(hc + 1) * 128, :])
                xin0.append(t)
                w = sb.tile([128, 512], F16, name=f"wc_{d}0{hc}",
                            tag="wc", bufs=20)
                nc.sync.dma_start(w[:], convw[d, 0, :, hc])
                wc[0][hc] = w
            for j in range(1, 4):
                for hc in range(HC):
                    w = sb.tile([128, 512], F16, name=f"wc_{d}{j}{hc}",
                                tag="wc", bufs=20)
                    nc.sync.dma_start(w[:], convw[d, j, :, hc])
                    wc[j][hc] = w
            wh = []
            for jh in range(NHW):
                row = []
                for gc in range(8):
                    t = sb.tile([128, HC, 128], F16,
                                name=f"wh_{d}{jh}{gc}", tag="wh", bufs=20)
                    nc.sync.dma_start(t[:], hww[d, jh, gc])
                    row.append(t)
                wh.append(row)
            cb = sb.tile([128, 4], F32, name=f"cb_{d}", tag="cb", bufs=2)
            nc.gpsimd.dma_start(cb[:], convb[0, d])
            hb = sb.tile([128, NHW * 8], F32, name=f"hb_{d}", tag="hb",
                         bufs=4)
            nc.gpsimd.dma_start(hb[:], hwb[0, d])
            # pre-write layer-1 pad columns into xmid (off critical path)
            padf = sb.tile([128, 2, 3], F8, name=f"padf_{d}",
                           tag="padf", bufs=2)
            padb = sb.tile([128, 2, 3], F8, name=f"padb_{d}",
                           tag="padb", bufs=2)
            for b in range(BL):
                for a in range(2):
                    if b == 0:
                        nc.gpsimd.dma_start(padf[:], padt8[0, a])
                        nc.gpsimd.dma_start(padb[:], padt8[1, a])
                    nc.gpsimd.dma_start(xmid[d][b, a, :, :, 0:WIDTH],
                                        padf[:])
                    nc.gpsimd.dma_start(
                        xmid[d][b, a, :, :, WIDTH + S:WIDTH + S + WIDTH],
                        padb[:])

            off = 0 if d == 0 else WIDTH

            for b in range(BL):
                xin = xin0 if b == 0 else load_xin(b)

                # conv: out[oc,pos] = relu(b + sum_{j,hc} wT[j,hc,oc] @ x[hc,pos+j])
                xcur = []
                for oc in range(4):
                    pt = ps.tile([128, 512], F32, name=f"cps_{d}{b}{oc}",
                                 tag="ps")
                    k = 0
                    for j in range(4):
                        for hc in range(HC):
                            nc.tensor.matmul(
                                pt[:],
                                wc[j][hc][:, oc * 128:(oc + 1) * 128],
                                xin[hc][:, off + j:off + j + S],
                                start=(k == 0), stop=(k == 15))
                            k += 1
                    h = sb.tile([128, 512], F16, name=f"hf_{d}{b}{oc}",
                                tag="hf", bufs=8)
                    nc.scalar.activation(h[:], pt[:], AF.Relu,
                                         bias=cb[:, oc:oc + 1])
                    xcur.append(h)

                # highway sublayers
                for jh in range(NHW):
                    rt = [None] * 4
                    gt = [None] * 4
                    for gc in (0, 4, 1, 5, 2, 6, 3, 7):
                        pt = ps.tile([128, 512], F32,
                                     name=f"hps_{d}{b}{jh}{gc}", tag="ps")
                        for hc in range(HC):
                            nc.tensor.matmul(
                                pt[:],
                                wh[jh][gc][:, hc, :],
                                xcur[hc][:],
                                start=(hc == 0), stop=(hc == HC - 1))
                        if gc < 4:
                            r = sb.tile([128, 512], F16,
                                        name=f"rt_{d}{b}{jh}{gc}",
                                        tag="rt", bufs=6)
                            nc.scalar.activation(r[:], pt[:], AF.Relu,
                                                 bias=hb[:, jh * 8 + gc:jh * 8 + gc + 1])
                            rt[gc] = r
                        else:
                            g = sb.tile([128, 512], F16,
                                        name=f"gt_{d}{b}{jh}{gc}",
                                        tag="gt", bufs=6)
                            nc.scalar.activation(g[:], pt[:], AF.Sigmoid,
                                                 bias=hb[:, jh * 8 + gc:jh * 8 + gc + 1])
                            gt[gc - 4] = g
                    xnew = []
                    for hc in range(HC):
                        xo = sb.tile([128, 512], F16,
                                     name=f"xo_{d}{b}{jh}{hc}",
                                     tag=f"xo{jh}", bufs=8)
                        nc.vector.tensor_sub(xo[:], xcur[hc][:], rt[hc][:])
                        nc.vector.tensor_mul(xo[:], gt[hc][:], xo[:])
                        nc.vector.tensor_add(xo[:], xo[:], rt[hc][:])
                        xnew.append(xo)
                    xcur = xnew

                for hc in range(HC):
                    # fp8 copy (scaled by XS) feeds layer 1's conv
                    x8 = sb.tile([128, 512], F8, name=f"x8_{d}{b}{hc}",
                                 tag="x8", bufs=8)
                    nc.vector.tensor_scalar_mul(x8[:], xcur[hc][:], XS)
                    nc.sync.dma_start(
                        xmid[d][b, hc // 2, :, hc % 2, WIDTH:WIDTH + S],
                        x8[:])
                    nc.gpsimd.dma_start(
                        out_t[0, d, b, hc * 128:(hc + 1) * 128, :],
                        xcur[hc][:])

        # ---------------- layer 1 (fp8 DoubleRow) ----------------
        for d in range(2):
            def load_xin8(b):
                tiles = []
                for a in range(2):
                    t = sb.tile([128, 2, SP8], F8, name=f"xin8_{d}{b}{a}",
                                tag="xin8", bufs=8)
                    nc.sync.dma_start(t[:], xmid[d][b, a])
                    tiles.append(t)
                return tiles

            wc8 = [[None] * 2 for _ in range(4)]
            for j in range(4):
                for a in range(2):
                    w = sb.tile([128, 2, 512], F8, name=f"wc8_{d}{j}{a}",
                                tag="wc8", bufs=10)
                    nc.sync.dma_start(w[:], convw8[d, j, a])
                    wc8[j][a] = w
            wh8 = []
            for jh in range(NHW):
                row = []
                for gc in range(8):
                    t = sb.tile([128, 2, 2, 128], F8,
                                name=f"wh8_{d}{jh}{gc}", tag="wh8", bufs=20)
                    nc.sync.dma_start(t[:], hww8[d, jh, gc])
                    row.append(t)
                wh8.append(row)
            cb = sb.tile([128, 4], F32, name=f"cb1_{d}", tag="cb", bufs=2)
            nc.gpsimd.dma_start(cb[:], convb[1, d])
            hb = sb.tile([128, NHW * 8], F32, name=f"hb1_{d}", tag="hb",
                         bufs=4)
            nc.gpsimd.dma_start(hb[:], hwb[1, d])

            off = 0 if d == 0 else WIDTH

            for b in range(BL):
                xin8 = load_xin8(b)

                # conv in DoubleRow fp8: PSUM holds CWS*XS*z
                xcur = []                 # fp16 passthrough tiles
                h8p = [None, None]        # fp8 pair tiles for highway rhs
                for oc in range(4):
                    pt = ps.tile([128, 512], F32, name=f"cps1_{d}{b}{oc}",
                                 tag="ps")
                    k = 0
                    for j in range(4):
                        for a in range(2):
                            nc.tensor.matmul(
                                pt[:],
                                wc8[j][a][:, :, oc * 128:(oc + 1) * 128],
                                xin8[a][:, :, off + j:off + j + S],
                                start=(k == 0), stop=(k == 7), perf_mode=DR)
                            k += 1
                    h = sb.tile([128, 512], F16, name=f"hf1_{d}{b}{oc}",
                                tag="hf", bufs=8)
                    nc.scalar.activation(h[:], pt[:], AF.Relu,
                                         bias=cb[:, oc:oc + 1],
                                         scale=1.0 / (CWS * XS))
                    xcur.append(h)
                    if h8p[oc // 2] is None:
                        h8p[oc // 2] = sb.tile(
                            [128, 2, 512], F8, name=f"h8_{d}{b}{oc // 2}",
                            tag="h8", bufs=4)
                    # fp8 copy on VectorE: keeps ScalarE at 20 ops/batch
                    # (it would otherwise gate the PE via PSUM-free stalls)
                    nc.vector.tensor_scalar_mul(h8p[oc // 2][:, oc % 2, :],
                                                h[:], XS)

                # highway sublayers in DoubleRow fp8 (PSUM = HWS*XS*proj)
                for jh in range(NHW):
                    rt = [None] * 4
                    gt = [None] * 4
                    for gc in (0, 4, 1, 5, 2, 6, 3, 7):
                        pt = ps.tile([128, 512], F32,
                                     name=f"hps1_{d}{b}{jh}{gc}", tag="ps")
                        for a in range(2):
                            nc.tensor.matmul(
                                pt[:],
                                wh8[jh][gc][:, a, :, :],
                                h8p[a][:],
                                start=(a == 0), stop=(a == 1), perf_mode=DR)
                        if gc < 4:
                            r = sb.tile([128, 512], F16,
                                        name=f"rt1_{d}{b}{jh}{gc}",
                                        tag="rt", bufs=6)
                            nc.scalar.activation(r[:], pt[:], AF.Relu,
                                                 bias=hb[:, jh * 8 + gc:jh * 8 + gc + 1],
                                                 scale=1.0 / (HWS * XS))
                            rt[gc] = r
                        else:
                            g = sb.tile([128, 512], F16,
                                        name=f"gt1_{d}{b}{jh}{gc}",
                                        tag="gt", bufs=6)
                            nc.scalar.activation(g[:], pt[:], AF.Sigmoid,
                                                 bias=hb[:, jh * 8 + gc:jh * 8 + gc + 1],
                                                 scale=1.0 / (HWS * XS))
                            gt[gc - 4] = g
                    xnew = []
                    h8p_new = [None, None]
                    for hc in range(HC):
                        xo = sb.tile([128, 512], F16,
                                     name=f"xo1_{d}{b}{jh}{hc}",
                                     tag=f"xo{jh}", bufs=8)
                        nc.vector.tensor_sub(xo[:], xcur[hc][:], rt[hc][:])
                        nc.vector.tensor_mul(xo[:], gt[hc][:], xo[:])
                        nc.vector.tensor_add(xo[:], xo[:], rt[hc][:])
                        xnew.append(xo)
                        if jh < NHW - 1:
                            if h8p_new[hc // 2] is None:
                                h8p_new[hc // 2] = sb.tile(
                                    [128, 2, 512], F8,
                                    name=f"x18_{d}{b}{hc // 2}",
                                    tag="h8", bufs=4)
                            nc.vector.tensor_scalar_mul(
                                h8p_new[hc // 2][:, hc % 2, :], xo[:], XS)
                    xcur = xnew
                    if jh < NHW - 1:
                        h8p = h8p_new

                for hc in range(HC):
                    nc.gpsimd.dma_start(
                        out_t[1, d, b, hc * 128:(hc + 1) * 128, :],
                        xcur[hc][:])

    nc.compile()
    _CACHE["nc"] = nc
    return nc


def _prep_shared(fwd_pad, bwd_pad, fwd_w, fwd_b, bwd_w, bwd_b,
                 fwd_hw_w, fwd_hw_b, bwd_hw_w, bwd_hw_b):
    f32 = np.float32
    convw = np.empty((2, 4, 128, HC, 512), np.float16)
    convw8 = np.empty((2, 4, 2, 128, 2, 512), E4NP)
    convb = np.empty((L, 2, 128, 4), f32)
    hww = np.empty((2, NHW, 8, 128, HC, 128), np.float16)
    hww8 = np.empty((2, NHW, 8, 128, 2, 2, 128), E4NP)
    hwb = np.empty((L, 2, 128, NHW * 8), f32)
    padt8 = np.empty((2, 2, 128, 2, 3), E4NP)
    for d, (w, bia, hw_w, hw_b, pad) in enumerate(
            [(fwd_w, fwd_b, fwd_hw_w, fwd_hw_b, None),
             (bwd_w, bwd_b, bwd_hw_w, bwd_hw_b, None)]):
        # layer 0 fp16: w[0]: [512o, 2048=(j,hc,p)] -> [j, p, hc, o]
        convw[d] = w[0].reshape(512, 4, HC, 128).transpose(1, 3, 2, 0)
        # layer 1 fp8 DoubleRow: [512o, (j,a,ko,ki)] -> [j, a, ki, ko, o]
        convw8[d] = (w[1].reshape(512, 4, 2, 2, 128)
                     .transpose(1, 2, 4, 3, 0) * CWS).astype(E4NP)
        for li in range(L):
            convb[li, d] = bia[li].reshape(4, 128).T
            for jh in range(NHW):
                hwb[li, d][:, jh * 8:(jh + 1) * 8] = \
                    hw_b[li, jh].reshape(8, 128).T
        for jh in range(NHW):
            # layer 0: hw_w[0,jh]: [1024=(gc,gi), 512=(hc,p)] -> [gc, p, hc, gi]
            hww[d, jh] = hw_w[0, jh].reshape(8, 128, HC, 128) \
                                    .transpose(0, 3, 2, 1)
            # layer 1 fp8: [(gc,m), (a,ko,ki)] -> [gc, ki, a, ko, m]
            hww8[d, jh] = (hw_w[1, jh].reshape(8, 128, 2, 2, 128)
                           .transpose(0, 4, 2, 3, 1) * HWS).astype(E4NP)
    # layer-1 pads, fp8 pair layout [f/b, a, ki, ko, 3], scaled by XS
    padt8[0] = (fwd_pad[1].T.reshape(2, 2, 128, 3).transpose(0, 2, 1, 3)
                * XS).astype(E4NP)
    padt8[1] = (bwd_pad[1].T.reshape(2, 2, 128, 3).transpose(0, 2, 1, 3)
                * XS).astype(E4NP)
    return dict(convw=convw, convw8=convw8, convb=convb,
                hww=hww, hww8=hww8, hwb=hwb, padt8=padt8)


def kernel(inputs, fwd_pad, bwd_pad, fwd_w, fwd_b, bwd_w, bwd_b,
           fwd_hw_w, fwd_hw_b, bwd_hw_w, bwd_hw_b, _trace=False):
    nc = _build()
    shared = _prep_shared(
        np.asarray(fwd_pad), np.asarray(bwd_pad),
        np.asarray(fwd_w), np.asarray(fwd_b),
        np.asarray(bwd_w), np.asarray(bwd_b),
        np.asarray(fwd_hw_w), np.asarray(fwd_hw_b),
        np.asarray(bwd_hw_w), np.asarray(bwd_hw_b))
    x = np.asarray(inputs, dtype=np.float32)

    in_maps = []
    for c in range(NCORES):
        xs = x[c * BL:(c + 1) * BL].transpose(0, 2, 1)  # [BL, H, S]
        xc = np.empty((BL, H, SP), np.float16)
        xc[:, :, WIDTH:WIDTH + S] = xs
        xc[:, :, 0:WIDTH] = np.asarray(fwd_pad)[0].T[None]
        xc[:, :, WIDTH + S:SP] = np.asarray(bwd_pad)[0].T[None]
        in_maps.append({"x_t": xc, **shared})

    res = bass_utils.run_bass_kernel_spmd(
        nc, in_maps, core_ids=list(range(NCORES)), trace=_trace)

    out = np.empty((L, B, S, 2 * H), np.float32)
    for c in range(NCORES):
        o = res.results[c]["out_t"].astype(np.float32)  # [L, 2, BL, H, S]
        for li in range(L):
            out[li, c * BL:(c + 1) * BL, :, :H] = o[li, 0].transpose(0, 2, 1)
            out[li, c * BL:(c + 1) * BL, :, H:] = o[li, 1].transpose(0, 2, 1)
    if _trace:
        kernel.last_exec_time_ns = res.exec_time_ns
        kernel.last_trace = (res.instructions_and_trace[1]
                             if res.instructions_and_trace else None)
    return out


# revision 18
# speedup vs baseline: 1.1266x; 1.1266x over previous
"""Trainium2 Bass kernel for Bengio03HighwayBiLm.

Model: L=2 layers x 2 directions of [width-4 conv over sequence (H=512 -> 512)
+ ReLU + 2 highway sublayers (512 -> 1024 split into nonlin/gate)].

Sharding: data-parallel over batch across 8 cores (4 batches/core), weights
replicated. On device everything runs in channels-on-partitions layout
([ch, pos]); the host pre-transposes the input / weights and post-transposes
the output, so the device kernel needs no transposes at all. The conv is 4
accumulated matmuls over a column-shifted padded activation window.

Precision: layer 0 runs in float16 (full PE rate). Layer 1 runs its matmuls
in float8e4 with perf_mode=DoubleRow (2 fp8 weights per PE cell -> 2x rate,
K=256 per instruction). fp8 operands carry power-of-2 scales (weights 2^9 /
2^8, activations 2^3) undone for free via the scalar-activation `scale`
operand; PSUM accumulates fp32. End-to-end rel-rms error ~1.2e-2 (layer-0
fp8 would blow the 2e-2 budget; layer-1-only keeps 40% margin).
ScalarE applies bias+ReLU / bias+Sigmoid straight out of PSUM; VectorE does
the 3-op highway combine x' = r + g*(x - r) plus the fp8 re-quant copies.
"""

import sys

for _p in ("/opt/trn_rl_repo", "/root/.axon_site/_ro/trn_rl_repo"):
    if _p not in sys.path:
        sys.path.append(_p)

from contextlib import ExitStack

import numpy as np
import ml_dtypes

import concourse.bass as bass
import concourse.tile as tile
from concourse import bacc, bass_utils, mybir

F32 = mybir.dt.float32
F16 = mybir.dt.float16
F8 = mybir.dt.float8e4
AF = mybir.ActivationFunctionType
DR = mybir.MatmulPerfMode.DoubleRow
E4NP = ml_dtypes.float8_e4m3

B, S, H = 32, 512, 512
L, NHW, WIDTH = 2, 2, 3
NCORES = 8
BL = B // NCORES          # batches per core
SP = S + 2 * WIDTH        # padded sequence length (fp16 layer-0 input)
SP8 = 528                 # fp8 padded length (DoubleRow needs stride % 16 == 0)
HC = H // 128             # channel chunks

# fp8 operand scales (powers of 2; undone in activation `scale`)
XS = 2.0 ** 3             # activations
CWS = 2.0 ** 9            # layer-1 conv weights
HWS = 2.0 ** 8            # layer-1 highway weights

_CACHE = {}


def _build():
    if "nc" in _CACHE:
        return _CACHE["nc"]

    nc = bacc.Bacc("TRN2", target_bir_lowering=False, debug=False,
                   num_devices=NCORES)

    x_t = nc.dram_tensor("x_t", [BL, H, SP], F16, kind="ExternalInput").ap()
    convw = nc.dram_tensor("convw", [2, 4, 128, HC, 512], F16,
                           kind="ExternalInput").ap()          # layer 0
    hww = nc.dram_tensor("hww", [2, NHW, 8, 128, HC, 128], F16,
                         kind="ExternalInput").ap()            # layer 0
    convw8 = nc.dram_tensor("convw8", [2, 4, 2, 128, 2, 512], F8,
                            kind="ExternalInput").ap()         # layer 1
    hww8 = nc.dram_tensor("hww8", [2, NHW, 8, 128, 2, 2, 128], F8,
                          kind="ExternalInput").ap()           # layer 1
    convb = nc.dram_tensor("convb", [L, 2, 128, 4], F32,
                           kind="ExternalInput").ap()
    convb8 = nc.dram_tensor("convb8", [2, 128, 4], F32,
                            kind="ExternalInput").ap()         # XS * convb[1]
    hwb8 = nc.dram_tensor("hwb8", [2, 128, NHW * 8], F32,
                          kind="ExternalInput").ap()           # HWS*XS*hwb[1]
    hwb = nc.dram_tensor("hwb", [L, 2, 128, NHW * 8], F32,
                         kind="ExternalInput").ap()
    padt8 = nc.dram_tensor("padt8", [2, 2, 128, 2, 3], F8,
                           kind="ExternalInput").ap()          # layer-1 pads
    out_t = nc.dram_tensor("out_t", [L, 2, BL, H, S], F16,
                           kind="ExternalOutput").ap()

    with tile.TileContext(nc) as tc, ExitStack() as ctx:
        sb = ctx.enter_context(tc.tile_pool(name="sb", bufs=2))
        ps = ctx.enter_context(tc.tile_pool(name="ps", bufs=8, space="PSUM"))
        dr = ctx.enter_context(tc.tile_pool(name="dr", bufs=1, space="DRAM"))

        # fp8 inter-layer activations, pre-paired for DoubleRow:
        # [b, pair, ki, ko, pos], channel = 128*(2*pair+ko)+ki, value = XS*x
        xmid = []
        for d in range(2):
            m = dr.tile([BL, 2, 128, 2, SP8], F8, name=f"xmid{d}",
                        tag=f"xmid{d}")
            xmid.append(m)

        # ---------------- layer 0 (fp16) ----------------
        for d in range(2):
            def load_xin(b):
                tiles = []
                for hc in range(HC):
                    t = sb.tile([128, SP], F16, name=f"xin_{d}{b}{hc}",
                                tag="xin", bufs=14)
                    nc.sync.dma_start(t[:], x_t[b, hc * 128:(hc + 1) * 128, :])
                    tiles.append(t)
                return tiles

            # interleave first-batch inputs with tap-0 weights so the
            # first matmul's deps land earliest in the DMA queues
            xin0 = []
            wc = [[None] * HC for _ in range(4)]
            for hc in range(HC):
                t = sb.tile([128, SP], F16, name=f"xin_{d}0{hc}",
                            tag="xin", bufs=14)
                nc.sync.dma_start(t[:], x_t[0, hc * 128:(hc + 1) * 128, :])
                xin0.append(t)
                w = sb.tile([128, 512], F16, name=f"wc_{d}0{hc}",
                            tag="wc", bufs=20)
                nc.sync.dma_start(w[:], convw[d, 0, :, hc])
                wc[0][hc] = w
            for j in range(1, 4):
                for hc in range(HC):
                    w = sb.tile([128, 512], F16, name=f"wc_{d}{j}{hc}",
                                tag="wc", bufs=20)
                    nc.sync.dma_start(w[:], convw[d, j, :, hc])
                    wc[j][hc] = w
            wh = []
            for jh in range(NHW):
                row = []
                for gc in range(8):
                    t = sb.tile([128, HC, 128], F16,
                                name=f"wh_{d}{jh}{gc}", tag="wh", bufs=20)
                    nc.sync.dma_start(t[:], hww[d, jh, gc])
                    row.append(t)
                wh.append(row)
            cb = sb.tile([128, 4], F32, name=f"cb_{d}", tag="cb", bufs=2)
            nc.gpsimd.dma_start(cb[:], convb[0, d])
            hb = sb.tile([128, NHW * 8], F32, name=f"hb_{d}", tag="hb",
                         bufs=4)
            nc.gpsimd.dma_start(hb[:], hwb[0, d])
            # pre-write layer-1 pad columns into xmid (off critical path)
            padf = sb.tile([128, 2, 3], F8, name=f"padf_{d}",
                           tag="padf", bufs=2)
            padb = sb.tile([128, 2, 3], F8, name=f"padb_{d}",
                           tag="padb", bufs=2)
            for b in range(BL):
                for a in range(2):
                    if b == 0:
                        nc.gpsimd.dma_start(padf[:], padt8[0, a])
                        nc.gpsimd.dma_start(padb[:], padt8[1, a])
                    nc.gpsimd.dma_start(xmid[d][b, a, :, :, 0:WIDTH],
                                        padf[:])
                    nc.gpsimd.dma_start(
                        xmid[d][b, a, :, :, WIDTH + S:WIDTH + S + WIDTH],
                        padb[:])

            off = 0 if d == 0 else WIDTH

            for b in range(BL):
                xin = xin0 if b == 0 else load_xin(b)

                # conv: out[oc,pos] = relu(b + sum_{j,hc} wT[j,hc,oc] @ x[hc,pos+j])
                xcur = []
                for oc in range(4):
                    pt = ps.tile([128, 512], F32, name=f"cps_{d}{b}{oc}",
                                 tag="ps")
                    k = 0
                    for j in range(4):
                        for hc in range(HC):
                            nc.tensor.matmul(
                                pt[:],
                                wc[j][hc][:, oc * 128:(oc + 1) * 128],
                                xin[hc][:, off + j:off + j + S],
                                start=(k == 0), stop=(k == 15))
                            k += 1
                    h = sb.tile([128, 512], F16, name=f"hf_{d}{b}{oc}",
                                tag="hf", bufs=8)
                    nc.scalar.activation(h[:], pt[:], AF.Relu,
                                         bias=cb[:, oc:oc + 1])
                    xcur.append(h)

                # highway sublayers
                for jh in range(NHW):
                    rt = [None] * 4
                    gt = [None] * 4
                    for gc in (0, 4, 1, 5, 2, 6, 3, 7):
                        pt = ps.tile([128, 512], F32,
                                     name=f"hps_{d}{b}{jh}{gc}", tag="ps")
                        for hc in range(HC):
                            nc.tensor.matmul(
                                pt[:],
                                wh[jh][gc][:, hc, :],
                                xcur[hc][:],
                                start=(hc == 0), stop=(hc == HC - 1))
                        if gc < 4:
                            r = sb.tile([128, 512], F16,
                                        name=f"rt_{d}{b}{jh}{gc}",
                                        tag="rt", bufs=6)
                            nc.scalar.activation(r[:], pt[:], AF.Relu,
                                                 bias=hb[:, jh * 8 + gc:jh * 8 + gc + 1])
                            rt[gc] = r
                        else:
                            g = sb.tile([128, 512], F16,
                                        name=f"gt_{d}{b}{jh}{gc}",
                                        tag="gt", bufs=6)
                            nc.scalar.activation(g[:], pt[:], AF.Sigmoid,
                                                 bias=hb[:, jh * 8 + gc:jh * 8 + gc + 1])
                            gt[gc - 4] = g
                    xnew = []
                    for hc in range(HC):
                        xo = sb.tile([128, 512], F16,
                                     name=f"xo_{d}{b}{jh}{hc}",
                                     tag=f"xo{jh}", bufs=8)
                        nc.vector.tensor_sub(xo[:], xcur[hc][:], rt[hc][:])
                        nc.vector.tensor_mul(xo[:], gt[hc][:], xo[:])
                        nc.vector.tensor_add(xo[:], xo[:], rt[hc][:])
                        xnew.append(xo)
                    xcur = xnew

                for hc in range(HC):
                    # fp8 copy (scaled by XS) feeds layer 1's conv
                    x8 = sb.tile([128, 512], F8, name=f"x8_{d}{b}{hc}",
                                 tag="x8", bufs=8)
                    nc.vector.tensor_scalar_mul(x8[:], xcur[hc][:], XS)
                    nc.sync.dma_start(
                        xmid[d][b, hc // 2, :, hc % 2, WIDTH:WIDTH + S],
                        x8[:])
                    nc.gpsimd.dma_start(
                        out_t[0, d, b, hc * 128:(hc + 1) * 128, :],
                        xcur[hc][:])

        # ---------------- layer 1 (fp8 DoubleRow) ----------------
        for d in range(2):
            def load_xin8(b):
                tiles = []
                for a in range(2):
                    t = sb.tile([128, 2, SP8], F8, name=f"xin8_{d}{b}{a}",
                                tag="xin8", bufs=8)
                    nc.sync.dma_start(t[:], xmid[d][b, a])
                    tiles.append(t)
                return tiles

            wc8 = [[None] * 2 for _ in range(4)]
            for j in range(4):
                for a in range(2):
                    w = sb.tile([128, 2, 512], F8, name=f"wc8_{d}{j}{a}",
                                tag="wc8", bufs=10)
                    nc.sync.dma_start(w[:], convw8[d, j, a])
                    wc8[j][a] = w
            wh8 = []
            for jh in range(NHW):
                row = []
                for gc in range(8):
                    t = sb.tile([128, 2, 2, 128], F8,
                                name=f"wh8_{d}{jh}{gc}", tag="wh8", bufs=20)
                    nc.sync.dma_start(t[:], hww8[d, jh, gc])
                    row.append(t)
                wh8.append(row)
            cb8 = sb.tile([128, 4], F32, name=f"cb8_{d}", tag="cb8", bufs=2)
            nc.gpsimd.dma_start(cb8[:], convb8[d])
            hb = sb.tile([128, NHW * 8], F32, name=f"hb1_{d}", tag="hb",
                         bufs=4)
            nc.gpsimd.dma_start(hb[:], hwb[1, d])
            hb8 = sb.tile([128, NHW * 8], F32, name=f"hb8_{d}", tag="hb8",
                          bufs=2)
            nc.gpsimd.dma_start(hb8[:], hwb8[d])

            off = 0 if d == 0 else WIDTH

            # layer 1 runs in the XS-scaled domain end to end: all fp16
            # activation tiles hold XS*value, and the host divides the
            # layer-1 output by XS (free). Engine split per batch keeps
            # every engine under the 13.8us tensor time: ScalarE 20 ops
            # (h8, h*, sigmoid, x18 cast), VectorE 32 (relu via
            # tensor_scalar + STT-rescaled combine).
            C = HWS * XS
            for b in range(BL):
                xin8 = load_xin8(b)

                # conv in DoubleRow fp8: PSUM holds CWS*XS*z
                xcur = []                 # fp16 XS-domain passthrough tiles
                h8p = [None, None]        # fp8 pair tiles for highway rhs
                for oc in range(4):
                    pt = ps.tile([128, 512], F32, name=f"cps1_{d}{b}{oc}",
                                 tag="ps")
                    k = 0
                    for j in range(4):
                        for a in range(2):
                            nc.tensor.matmul(
                                pt[:],
                                wc8[j][a][:, :, oc * 128:(oc + 1) * 128],
                                xin8[a][:, :, off + j:off + j + S],
                                start=(k == 0), stop=(k == 7), perf_mode=DR)
                            k += 1
                    if h8p[oc // 2] is None:
                        h8p[oc // 2] = sb.tile(
                            [128, 2, 512], F8, name=f"h8_{d}{b}{oc // 2}",
                            tag="h8", bufs=4)
                    # XS*h = relu(psum/CWS + XS*b); fp8 copy first -- it
                    # feeds the next matmuls
                    nc.scalar.activation(h8p[oc // 2][:, oc % 2, :], pt[:],
                                         AF.Relu, bias=cb8[:, oc:oc + 1],
                                         scale=1.0 / CWS)
                    h = sb.tile([128, 512], F16, name=f"hf1_{d}{b}{oc}",
                                tag="hf", bufs=8)
                    nc.scalar.activation(h[:], pt[:], AF.Relu,
                                         bias=cb8[:, oc:oc + 1],
                                         scale=1.0 / CWS)
                    xcur.append(h)

                # highway sublayers in DoubleRow fp8 (PSUM = HWS*XS*proj)
                for jh in range(NHW):
                    rt = [None] * 4       # r'' = C*XS*relu(proj+b), fp16
                    gt = [None] * 4
                    for gc in (0, 4, 1, 5, 2, 6, 3, 7):
                        pt = ps.tile([128, 512], F32,
                                     name=f"hps1_{d}{b}{jh}{gc}", tag="ps")
                        for a in range(2):
                            nc.tensor.matmul(
                                pt[:],
                                wh8[jh][gc][:, a, :, :],
                                h8p[a][:],
                                start=(a == 0), stop=(a == 1), perf_mode=DR)
                        bi = jh * 8 + gc
                        if gc < 4:
                            # VectorE: r'' = max(psum + C*hb, 0)  (C-scaled)
                            r = sb.tile([128, 512], F16,
                                        name=f"rt1_{d}{b}{jh}{gc}",
                                        tag="rt", bufs=6)
                            nc.vector.tensor_scalar(
                                r[:], pt[:], hb8[:, bi:bi + 1], 0.0,
                                mybir.AluOpType.add, mybir.AluOpType.max)
                            rt[gc] = r
                        else:
                            g = sb.tile([128, 512], F16,
                                        name=f"gt1_{d}{b}{jh}{gc}",
                                        tag="gt", bufs=6)
                            nc.scalar.activation(g[:], pt[:], AF.Sigmoid,
                                                 bias=hb[:, bi:bi + 1],
                                                 scale=1.0 / C)
                            gt[gc - 4] = g
                    xnew = []
                    h8p_new = [None, None]
                    for hc in range(HC):
                        # x1* = XS*r + g*(XS*h - XS*r), with r'' = C*r:
                        #   t  = r''*(XS/C) - h*     (= XS*(r-h))
                        #   t  = g*t
                        #   x1* = r''*(XS/C) - t
                        xo = sb.tile([128, 512], F16,
                                     name=f"xo1_{d}{b}{jh}{hc}",
                                     tag=f"xo{jh}", bufs=8)
                        nc.vector.scalar_tensor_tensor(
                            xo[:], rt[hc][:], XS / C, xcur[hc][:],
                            mybir.AluOpType.mult, mybir.AluOpType.subtract)
                        nc.vector.tensor_mul(xo[:], gt[hc][:], xo[:])
                        nc.vector.scalar_tensor_tensor(
                            xo[:], rt[hc][:], XS / C, xo[:],
                            mybir.AluOpType.mult, mybir.AluOpType.subtract)
                        xnew.append(xo)
                        if jh < NHW - 1:
                            if h8p_new[hc // 2] is None:
                                h8p_new[hc // 2] = sb.tile(
                                    [128, 2, 512], F8,
                                    name=f"x18_{d}{b}{hc // 2}",
                                    tag="h8", bufs=4)
                            # ScalarE: fp8 cast (values already XS-scaled)
                            nc.scalar.activation(
                                h8p_new[hc // 2][:, hc % 2, :], xo[:],
                                AF.Copy)
                    xcur = xnew
                    if jh < NHW - 1:
                        h8p = h8p_new

                for hc in range(HC):
                    nc.gpsimd.dma_start(
                        out_t[1, d, b, hc * 128:(hc + 1) * 128, :],
                        xcur[hc][:])

    nc.compile()
    _CACHE["nc"] = nc
    return nc


def _prep_shared(fwd_pad, bwd_pad, fwd_w, fwd_b, bwd_w, bwd_b,
                 fwd_hw_w, fwd_hw_b, bwd_hw_w, bwd_hw_b):
    f32 = np.float32
    convw = np.empty((2, 4, 128, HC, 512), np.float16)
    convw8 = np.empty((2, 4, 2, 128, 2, 512), E4NP)
    convb = np.empty((L, 2, 128, 4), f32)
    convb8 = np.empty((2, 128, 4), f32)
    hwb8 = np.empty((2, 128, NHW * 8), f32)
    hww = np.empty((2, NHW, 8, 128, HC, 128), np.float16)
    hww8 = np.empty((2, NHW, 8, 128, 2, 2, 128), E4NP)
    hwb = np.empty((L, 2, 128, NHW * 8), f32)
    padt8 = np.empty((2, 2, 128, 2, 3), E4NP)
    for d, (w, bia, hw_w, hw_b, pad) in enumerate(
            [(fwd_w, fwd_b, fwd_hw_w, fwd_hw_b, None),
             (bwd_w, bwd_b, bwd_hw_w, bwd_hw_b, None)]):
        # layer 0 fp16: w[0]: [512o, 2048=(j,hc,p)] -> [j, p, hc, o]
        convw[d] = w[0].reshape(512, 4, HC, 128).transpose(1, 3, 2, 0)
        # layer 1 fp8 DoubleRow: [512o, (j,a,ko,ki)] -> [j, a, ki, ko, o]
        convw8[d] = (w[1].reshape(512, 4, 2, 2, 128)
                     .transpose(1, 2, 4, 3, 0) * CWS).astype(E4NP)
        for li in range(L):
            convb[li, d] = bia[li].reshape(4, 128).T
            for jh in range(NHW):
                hwb[li, d][:, jh * 8:(jh + 1) * 8] = \
                    hw_b[li, jh].reshape(8, 128).T
        convb8[d] = convb[1, d] * XS
        hwb8[d] = hwb[1, d] * (HWS * XS)
        for jh in range(NHW):
            # layer 0: hw_w[0,jh]: [1024=(gc,gi), 512=(hc,p)] -> [gc, p, hc, gi]
            hww[d, jh] = hw_w[0, jh].reshape(8, 128, HC, 128) \
                                    .transpose(0, 3, 2, 1)
            # layer 1 fp8: [(gc,m), (a,ko,ki)] -> [gc, ki, a, ko, m]
            hww8[d, jh] = (hw_w[1, jh].reshape(8, 128, 2, 2, 128)
                           .transpose(0, 4, 2, 3, 1) * HWS).astype(E4NP)
    # layer-1 pads, fp8 pair layout [f/b, a, ki, ko, 3], scaled by XS
    padt8[0] = (fwd_pad[1].T.reshape(2, 2, 128, 3).transpose(0, 2, 1, 3)
                * XS).astype(E4NP)
    padt8[1] = (bwd_pad[1].T.reshape(2, 2, 128, 3).transpose(0, 2, 1, 3)
                * XS).astype(E4NP)
    return dict(convw=convw, convw8=convw8, convb=convb, convb8=convb8,
                hww=hww, hww8=hww8, hwb=hwb, hwb8=hwb8, padt8=padt8)


def kernel(inputs, fwd_pad, bwd_pad, fwd_w, fwd_b, bwd_w, bwd_b,
           fwd_hw_w, fwd_hw_b, bwd_hw_w, bwd_hw_b, _trace=False):
    nc = _build()
    shared = _prep_shared(
        np.asarray(fwd_pad), np.asarray(bwd_pad),
        np.asarray(fwd_w), np.asarray(fwd_b),
        np.asarray(bwd_w), np.asarray(bwd_b),
        np.asarray(fwd_hw_w), np.asarray(fwd_hw_b),
        np.asarray(bwd_hw_w), np.asarray(bwd_hw_b))
    x = np.asarray(inputs, dtype=np.float32)

    in_maps = []
    for c in range(NCORES):
        xs = x[c * BL:(c + 1) * BL].transpose(0, 2, 1)  # [BL, H, S]
        xc = np.empty((BL, H, SP), np.float16)
        xc[:, :, WIDTH:WIDTH + S] = xs
        xc[:, :, 0:WIDTH] = np.asarray(fwd_pad)[0].T[None]
        xc[:, :, WIDTH + S:SP] = np.asarray(bwd_pad)[0].T[None]
        in_maps.append({"x_t": xc, **shared})

    res = bass_utils.run_bass_kernel_spmd(
        nc, in_maps, core_ids=list(range(NCORES)), trace=_trace)

    out = np.empty((L, B, S, 2 * H), np.float32)
    for c in range(NCORES):
        o = res.results[c]["out_t"].astype(np.float32)  # [L, 2, BL, H, S]
        o[1] /= XS     # layer 1 is computed in the XS-scaled domain
        for li in range(L):
            out[li, c * BL:(c + 1) * BL, :, :H] = o[li, 0].transpose(0, 2, 1)
            out[li, c * BL:(c + 1) * BL, :, H:] = o[li, 1].transpose(0, 2, 1)
    if _trace:
        kernel.last_exec_time_ns = res.exec_time_ns
        kernel.last_trace = (res.instructions_and_trace[1]
                             if res.instructions_and_trace else None)
    return out
